# revision 15
# baseline (speedup 1.0000x reference)
"""Trainium2 Bass kernel for nn_CascadedAttention_76836964925817.

Math: the reference module's attention machinery is dead code — softmax over a
size-1 axis is identically 1, so `context = x[0].sum(axis=0)` is a constant
and the layer reduces to the 28-dim nonlinear recurrence

    y[t] = sigmoid(Wo @ y[t-1] + Uo @ x[t-1] + c),   c = Co @ sum_t x[t],
    y[-1] = 0, x[-1] := 0.

Strategy:
  * Precompute B[t] = Uo @ x[t-1] (a (2048, 28) matrix) and c on device.
    This phase is sharded over T across the 8 cores (each core handles 256
    timesteps of x, pre-transposed on the host so the contraction dim D lands
    on SBUF partitions), then an AllGather shares the per-core (28 x 256)
    results + partial c sums with every core.
  * Solve the recurrence by fixed-point (Jacobi) iteration:
        Y <- sigmoid(shift(Y) @ Wo.T + B + c)
    The map is a strong contraction (|sigmoid'| <= 1/4, ||Wo|| ~ 0.53;
    empirically the error floor is reached after 2-3 sweeps); N_ITERS sweeps
    are run, fully parallel over t on the tensor + scalar engines.
  * Layout for the iteration: t is split into 4 column groups of 512 on
    partition blocks 32g..32g+27, so every engine instruction runs with
    ~128 active partitions.  All matmuls contract over the full 128
    partitions with zero-masked weights so each group's accumulation chain
    stays at one PE tile position (HW: accumulation groups cannot span
    row-group tile positions).  Cross-group boundary terms (y[512g-1]) are
    injected by an extra N=1 matmul whose masked weight reads the previous
    group's partition rows.

The kernel is self-contained: shapes/sharding are hardcoded.
"""

import numpy as np

import concourse.bass as bass
import concourse.mybir as mybir
import concourse.tile as tile
from concourse import bacc
from concourse import bass_utils

F32 = mybir.dt.float32
F32R = mybir.dt.float32r
AF = mybir.ActivationFunctionType

T, D, V = 2048, 1024, 28
N_CORES = 8
TC = T // N_CORES          # 256 timesteps per core in the B-precompute phase
G = 4                      # column groups in the iteration phase
S = T // G                 # 512 columns per group
DCH = D // 128             # 8 contraction chunks
N_ITERS = 5                # fixed-point refinement sweeps (after the init sweep)
W2 = 64                    # padded [Uo;Co] output rows: Uo 0:28, Co 32:60

USE_F32R = False
USE_CC = True              # AllGather on; off = single-core-data debug mode


def r32(ap):
    return ap.bitcast(F32R) if USE_F32R else ap


def build_body(nc, xt, w2t, wg, wbnd, eyeg, yg, n_iters=N_ITERS, tc=None,
               reps=1):
    """Emit the program. xt:(1024,256) this core's x chunk transposed;
    w2t:(1024,64) zero-padded [Uo;Co].T; wg/wbnd/eyeg:(128,4,28) masked
    weights; yg:(128,512) grouped output.  reps>1 re-emits the whole body
    serially for device-time measurement."""
    t = tc
    from contextlib import ExitStack
    ctx = ExitStack()
    sbp = ctx.enter_context(t.tile_pool(name="sb", bufs=1))
    pp = ctx.enter_context(t.tile_pool(name="pp", bufs=1, space="PSUM"))
    dp = ctx.enter_context(t.tile_pool(name="dp", bufs=2, space="DRAM"))

    def st(shape, name):
        return sbp.tile(shape, F32, name=name, tag=name)

    xt_sb = st([128, DCH, TC], "xt_sb")
    w2t_sb = st([128, DCH, W2], "w2t_sb")
    wg_sb = st([128, G, V], "wg_sb")
    wbnd_sb = st([128, G, V], "wbnd_sb")
    eyeg_sb = st([128, G, V], "eyeg_sb")
    usb = st([W2, TC], "usb")
    cpart = st([W2, 1], "cpart")
    csb = st([V, N_CORES], "csb")
    ctmp = st([V, 1], "ctmp")
    crep = st([128, 1], "crep")
    bg = st([128, S], "bg")
    ya = st([128, S], "ya")
    dummy = st([1, 1], "dummy")

    upsum = pp.tile([W2, TC], F32, name="upsum", tag="upsum")
    psa = pp.tile([128, S], F32, name="psa", tag="psa")
    psb = pp.tile([128, S], F32, name="psb", tag="psb")

    # Early dummy sigmoid so the ACT table load happens off the critical path.
    nc.vector.memset(dummy[:, :], 0.0)
    nc.scalar.activation(out=dummy[:, :], in_=dummy[:, :], func=AF.Sigmoid)

    # one-time constants
    nc.sync.dma_start(wg_sb[:, :, :], wg)
    nc.sync.dma_start(wbnd_sb[:, :, :], wbnd)
    nc.sync.dma_start(eyeg_sb[:, :, :], eyeg)
    nc.sync.dma_start(w2t_sb[:, :, :], w2t.rearrange("(c p) v -> p c v", p=128))
    nc.vector.memset(crep[:, :], 0.0)
    nc.vector.memset(bg[:, :], 0.0)
    nc.vector.memset(psa[:, :], 0.0)
    nc.vector.memset(psb[:, :], 0.0)

    prev_last = None
    for _rep in range(reps):
        prev_last = emit_rep(nc, t, dp, xt, yg, n_iters,
                             xt_sb, w2t_sb, wg_sb, wbnd_sb, eyeg_sb, usb,
                             cpart, csb, ctmp, crep, bg, ya, upsum, psa, psb,
                             prev_last)
    ctx.close()


def emit_rep(nc, t, dp, xt, yg, n_iters,
             xt_sb, w2t_sb, wg_sb, wbnd_sb, eyeg_sb, usb, cpart, csb,
             ctmp, crep, bg, ya, upsum, psa, psb, prev_last=None):
    from concourse.tile_rust import add_dep_helper
    pay = dp.tile([V, TC + 1], F32, name="pay", tag="pay")
    agout = dp.tile([V * N_CORES, TC + 1], F32, name="agout", tag="agout",
                    addr_space="Shared")

    # ---------------- load x chunk ----------------
    xdma = nc.sync.dma_start(xt_sb[:, :, :],
                             xt.rearrange("(c p) t -> p c t", p=128))
    if prev_last is not None:
        add_dep_helper(xdma.ins, prev_last.ins,
                       reason="serialize reps for latency measurement")

    # ---------------- U = [Uo;Co] @ x_chunk.T  -> (64, 256) ----------------
    for c in range(DCH):
        nc.tensor.matmul(
            upsum[:, :],
            lhsT=r32(w2t_sb[:, c, :]),
            rhs=r32(xt_sb[:, c, :]),
            start=(c == 0),
            stop=(c == DCH - 1),
        )
    nc.vector.tensor_copy(usb[:, :], upsum[:, :])
    # partial c: row-sums of the Co part
    nc.vector.tensor_reduce(
        out=cpart[32:32 + V, :], in_=upsum[32:32 + V, :],
        axis=mybir.AxisListType.X, op=mybir.AluOpType.add,
    )

    # ---------------- AllGather U chunks + partial c ----------------
    nc.sync.dma_start(pay[0:V, 0:TC], usb[0:V, :])
    nc.sync.dma_start(pay[0:V, TC:TC + 1], cpart[32:32 + V, :])
    if USE_CC:
        nc.gpsimd.collective_compute(
            "AllGather",
            mybir.AluOpType.bypass,
            replica_groups=[list(range(N_CORES))],
            ins=[pay.opt()],
            outs=[agout.opt()],
        )
    else:
        nc.sync.dma_start(agout[0:V, :], pay[:, :])

    # ---------------- assemble grouped B and c ----------------
    # bg[32g+v, tau] = U[512g + tau - 1, v],  U[-1] = 0
    # U[t, v] lives at agout[28*(t//256) + v, t % 256]
    for g in range(G):
        sb = 32 * g
        r0, r1 = 2 * g, 2 * g + 1
        nc.sync.dma_start(bg[sb:sb + V, 1:TC + 1],
                          agout[V * r0:V * r0 + V, 0:TC])
        nc.sync.dma_start(bg[sb:sb + V, TC + 1:S],
                          agout[V * r1:V * r1 + V, 0:TC - 1])
        if g > 0:
            rb = 2 * g - 1
            nc.sync.dma_start(bg[sb:sb + V, 0:1],
                              agout[V * rb:V * rb + V, TC - 1:TC])

    # c = sum over cores of partial c; replicate to all 4 partition groups
    nc.sync.dma_start(
        csb[:, :],
        agout.opt().rearrange("(r p) f -> p r f", p=V)[0:V, :, TC:TC + 1],
    )
    nc.vector.tensor_reduce(out=ctmp[:, :], in_=csb[:, :],
                            axis=mybir.AxisListType.X, op=mybir.AluOpType.add)
    for g in range(G):
        nc.sync.dma_start(crep[32 * g:32 * g + V, :], ctmp[:, :])

    # ---------------- fixed-point iterations ----------------
    # psum[32g+v, tau] accumulates Wo @ y[512g+tau-1] + U[512g+tau-1];
    # ACT applies sigmoid(. + c) and writes y back into ya.
    for k in range(n_iters + 1):
        ps = psa if k % 2 == 0 else psb
        for g in range(G):
            base = 32 * g
            nc.tensor.matmul(
                ps[base:base + V, 0:S],
                lhsT=r32(eyeg_sb[:, g, :]),
                rhs=r32(bg[:, 0:S]),
                start=True, stop=(k == 0),
                tile_position=(0, base),
            )
            if k > 0:
                nc.tensor.matmul(
                    ps[base:base + V, 1:S],
                    lhsT=r32(wg_sb[:, g, :]),
                    rhs=r32(ya[:, 0:S - 1]),
                    start=False, stop=(g == 0),
                    tile_position=(0, base),
                )
                if g > 0:
                    nc.tensor.matmul(
                        ps[base:base + V, 0:1],
                        lhsT=r32(wbnd_sb[:, g, :]),
                        rhs=r32(ya[:, S - 1:S]),
                        start=False, stop=True,
                        tile_position=(0, base),
                    )
        nc.scalar.activation(out=ya[:, :], in_=ps[:, :], func=AF.Sigmoid,
                             bias=crep[:, 0:1], scale=1.0)

    # ---------------- write grouped output ----------------
    return nc.sync.dma_start(yg, ya[:, :])


_CACHED_NC = {}


def _get_nc(reps=1):
    if reps not in _CACHED_NC:
        nc = bacc.Bacc("TRN2", target_bir_lowering=False, debug=False,
                       num_devices=N_CORES)
        xt = nc.dram_tensor("xt", [D, TC], F32, kind="ExternalInput")
        w2t = nc.dram_tensor("w2t", [D, W2], F32, kind="ExternalInput")
        wg = nc.dram_tensor("wg", [128, G, V], F32, kind="ExternalInput")
        wbnd = nc.dram_tensor("wbnd", [128, G, V], F32, kind="ExternalInput")
        eyeg = nc.dram_tensor("eyeg", [128, G, V], F32, kind="ExternalInput")
        yg = nc.dram_tensor("yg", [128, S], F32, kind="ExternalOutput")
        with tile.TileContext(nc) as t:
            build_body(nc, xt.ap(), w2t.ap(), wg.ap(), wbnd.ap(), eyeg.ap(),
                       yg.ap(), tc=t, reps=reps)
        nc.compile()
        _CACHED_NC[reps] = nc
    return _CACHED_NC[reps]


def make_in_maps(x, Uo, Co, Wo):
    xb = np.ascontiguousarray(np.asarray(x, np.float32)[0])        # (T, D)
    w2 = np.zeros((W2, D), np.float32)
    w2[0:V] = np.asarray(Uo, np.float32)
    w2[32:32 + V] = np.asarray(Co, np.float32)
    w2t = np.ascontiguousarray(w2.T)                               # (D, 64)
    wot1 = np.ascontiguousarray(np.asarray(Wo, np.float32).T)      # (V, V)
    wg = np.zeros((128, G, V), np.float32)
    wbnd = np.zeros((128, G, V), np.float32)
    eyeg = np.zeros((128, G, V), np.float32)
    for g in range(G):
        wg[32 * g:32 * g + V, g, :] = wot1
        eyeg[32 * g:32 * g + V, g, :] = np.eye(V, dtype=np.float32)
        if g > 0:
            wbnd[32 * (g - 1):32 * (g - 1) + V, g, :] = wot1
    in_maps = []
    for r in range(N_CORES):
        xt_r = np.ascontiguousarray(xb[r * TC:(r + 1) * TC, :].T)  # (D, TC)
        in_maps.append({"xt": xt_r, "w2t": w2t, "wg": wg, "wbnd": wbnd,
                        "eyeg": eyeg})
    return in_maps


def unshard_output(yg):
    y = np.empty((T, V), np.float32)
    for g in range(G):
        y[g * S:(g + 1) * S, :] = yg[32 * g:32 * g + V, :].T
    return y[None]


def run(inputs, trace=False, reps=1, **kw):
    nc = _get_nc(reps)
    in_maps = make_in_maps(inputs["x"], inputs["Uo"], inputs["Co"],
                           inputs["Wo"])
    res = bass_utils.run_bass_kernel_spmd(
        nc, in_maps, core_ids=list(range(N_CORES)), trace=trace, **kw)
    return unshard_output(res.results[0]["yg"]), res


def kernel(**inputs):
    out, _ = run(inputs)
    return out


# revision 29
# speedup vs baseline: 5.3630x; 5.3630x over previous
"""Trainium2 Bass kernel for nn_CascadedAttention_76836964925817.

Math: the reference module's attention machinery is dead code — softmax over a
size-1 axis is identically 1, so `context = x[0].sum(axis=0)` is a constant
and the layer reduces to the 28-dim nonlinear recurrence

    y[t] = sigmoid(Wo @ y[t-1] + Uo @ x[t-1] + c),   c = Co @ sum_t x[t],
    y[-1] = 0, x[-1] := 0.

Strategy:
  * Precompute B[t] = Uo @ x[t-1] (a (2048, 28) matrix) and c on device.
    This phase is sharded over T across the 8 cores (each core handles 256
    timesteps of x, pre-transposed/interleaved on the host so the contraction
    dim D lands on SBUF partitions with one fully-contiguous DMA), then an
    AllGather shares the per-core (28 x 256) results + partial c sums.
  * Solve the recurrence by fixed-point (Jacobi) iteration:
        Y <- sigmoid(shift(Y) @ Wo.T + B + c)
    The map is a strong contraction (|sigmoid'| <= 1/4, ||Wo|| ~ 0.53;
    empirically the error floor is reached after 2-3 sweeps).
  * Iteration layout: t is split into 4 column groups of 512 stacked on
    partition blocks 28g..28g+27 (112 active partitions).  Each sweep is one
    three-matmul accumulation chain in fp32r (1 cycle/column on the PE):
        MM1: psum  = I112 @ bg                         (B term; bg pre-shifted)
        MM2: psum += blockdiag(Wo.T) @ YA[:, 0:512]    (shifted-y storage)
        MM3: psum += shiftblk(Wo.T) @ YA[:, 512:514]   (group boundary;
             col 513 is a permanent zero so the 2-col dst stays fp32r-legal)
    then one 112-lane sigmoid ACT with per-partition bias c writes
    YA[:, 1:513].  fp32r dst rules (start partition 0, even column count,
    8B alignment) hold by construction; masks are zero-padded host weights.

The kernel is self-contained: shapes/sharding are hardcoded.
"""

import numpy as np

import concourse.bass as bass
import concourse.mybir as mybir
import concourse.tile as tile
from concourse import bacc
from concourse import bass_utils

F32 = mybir.dt.float32
F32R = mybir.dt.float32r
BF16 = mybir.dt.bfloat16
AF = mybir.ActivationFunctionType

T, D, V = 2048, 1024, 28
N_CORES = 8
TC = T // N_CORES          # 256 timesteps per core in the B-precompute phase
G = 4                      # column groups in the iteration phase
S = T // G                 # 512 columns per group
P4 = G * V                 # 112 active partitions in the iteration phase
DCH = D // 128             # 8 contraction chunks
N_ITERS = 3                # fixed-point refinement sweeps (after the init sweep)
W2 = 64                    # padded [Uo;Co] output rows: Uo 0:28, Co 32:60
TH = TC + 2                # per-core timestep window incl. 2-col halo (even)

USE_F32R = True
USE_CC = True              # AllGather on; off = single-core-data debug mode


def build_body(nc, xt, w2t, wmm, eye, yg, n_iters=N_ITERS, tc=None,
               reps=1):
    """Emit the program. xt:(128, 8*256) x chunk, d-major interleaved;
    w2t:(1024,64) zero-padded [Uo;Co].T; wmm:(112, 3, 112) block weights
    ([.,0,.]=I112, [.,1,.]=blockdiag(Wo.T), [.,2,.]=boundary-shift(Wo.T));
    yg:(112,512) grouped output."""
    t = tc
    from contextlib import ExitStack
    ctx = ExitStack()
    sbp = ctx.enter_context(t.tile_pool(name="sb", bufs=1))
    pp = ctx.enter_context(t.tile_pool(name="pp", bufs=1, space="PSUM"))
    dp = ctx.enter_context(t.tile_pool(name="dp", bufs=2, space="DRAM"))

    MDT = F32R if USE_F32R else F32

    def st(shape, name, dt=F32):
        return sbp.tile(shape, dt, name=name, tag=name)

    xt_sb = st([128, 2, DCH, TH], "xt_sb", BF16)
    w2t_sb = st([128, 2, DCH, W2], "w2t_sb", BF16)
    wmm_sb = st([P4, 2, P4], "wmm_sb", MDT)
    eye_sb = st([P4, P4], "eye_sb", BF16)
    usb = st([W2, 2, TH], "usb", BF16)
    cpart = st([W2, 1], "cpart")
    cprt_bf = st([W2, 2], "cprt_bf", BF16)
    csb = st([P4, 2 * N_CORES], "csb", BF16)
    cbias = st([P4, 1], "cbias")
    bg = st([P4, 2, S], "bg", BF16)
    ya = st([P4, S + 2], "ya", MDT)
    yfin = st([P4, S], "yfin")
    dummy = st([1, 1], "dummy")

    upsum = pp.tile([W2, TH], F32, name="upsum", tag="upsum")
    psa = pp.tile([P4, S], F32, name="psa", tag="psa")
    psb = pp.tile([P4, S], F32, name="psb", tag="psb")

    # Early dummy sigmoid so the ACT table load happens off the critical path.
    nc.vector.memset(dummy[:, :], 0.0)
    nc.scalar.activation(out=dummy[:, :], in_=dummy[:, :], func=AF.Sigmoid)

    # one-time constants
    nc.sync.dma_start(wmm_sb[:, :, :], wmm)
    nc.sync.dma_start(eye_sb[:, :], eye)
    nc.sync.dma_start(w2t_sb[:, :, :, :],
                      w2t.rearrange("p (h c v) -> p h c v", h=2, c=DCH))
    nc.vector.memset(bg[:, :, :].bitcast(mybir.dt.uint16), 0)
    nc.vector.memset(ya[:, :].bitcast(F32), 0.0)

    prev_last = None
    for _rep in range(reps):
        prev_last = emit_rep(nc, t, dp, xt, yg, n_iters,
                             xt_sb, w2t_sb, wmm_sb, eye_sb, usb,
                             cpart, cprt_bf, csb, cbias, bg, ya, yfin,
                             upsum, psa, psb, prev_last)
    ctx.close()


def emit_rep(nc, t, dp, xt, yg, n_iters,
             xt_sb, w2t_sb, wmm_sb, eye_sb, usb, cpart, cprt_bf, csb,
             cbias, bg, ya, yfin, upsum, psa, psb, prev_last=None):
    from concourse.tile_rust import add_dep_helper
    MDT = F32R if USE_F32R else F32
    pay = dp.tile([V, 2 * TH + 2], BF16, name="pay", tag="pay")
    agout = dp.tile([V * N_CORES, 2 * TH + 2], BF16, name="agout",
                    tag="agout", addr_space="Shared")

    # ---------------- load x chunk (one fully-contiguous 1MB DMA) ----------
    xdma = nc.sync.dma_start(xt_sb[:, :, :, :],
                             xt.rearrange("p (h c t) -> p h c t", h=2, c=DCH))
    if prev_last is not None:
        add_dep_helper(xdma.ins, prev_last.ins,
                       reason="serialize reps for latency measurement")

    # -------- U = [Uo;Co] @ x_chunk.T  -> (64, 258), bf16 hi/lo split ------
    terms = [(0, 0), (0, 1), (1, 0)]   # (w half, x half); lo*lo dropped
    nmm = DCH * len(terms)
    i = 0
    for c in range(DCH):
        for hw, hx in terms:
            i += 1
            nc.tensor.matmul(
                upsum[:, :],
                lhsT=w2t_sb[:, hw, c, :],
                rhs=xt_sb[:, hx, c, :],
                start=(i == 1),
                stop=(i == nmm),
            )
    nc.vector.tensor_copy(usb[:, 0, :], upsum[:, :])
    nc.vector.tensor_tensor(usb[:, 1, :], upsum[:, :], usb[:, 0, :],
                            mybir.AluOpType.subtract)
    # partial c: row-sums of the Co part (own timesteps only, not the halo)
    nc.vector.tensor_reduce(
        out=cpart[32:32 + V, :], in_=upsum[32:32 + V, 2:TH],
        axis=mybir.AxisListType.X, op=mybir.AluOpType.add,
    )
    nc.vector.tensor_copy(cprt_bf[32:32 + V, 0:1], cpart[32:32 + V, :])
    nc.vector.tensor_tensor(cprt_bf[32:32 + V, 1:2], cpart[32:32 + V, :],
                            cprt_bf[32:32 + V, 0:1],
                            mybir.AluOpType.subtract)

    # ---------------- AllGather U chunks + partial c ----------------
    nc.sync.dma_start(pay[0:V, 0:2 * TH], usb[0:V, :, :])
    nc.sync.dma_start(pay[0:V, 2 * TH:2 * TH + 2], cprt_bf[32:32 + V, :])
    if USE_CC:
        nc.gpsimd.collective_compute(
            "AllGather",
            mybir.AluOpType.bypass,
            replica_groups=[list(range(N_CORES))],
            ins=[pay.opt()],
            outs=[agout.opt()],
        )
    else:
        nc.sync.dma_start(agout[0:V, :], pay[:, :])

    # ---------------- assemble grouped B and c ----------------
    # bg[28g+v, tau] = U[512g + tau - 1, v].  Core r's payload col j holds
    # U[256r - 2 + j] (2-col halo, core 0's halo is zero), so group g is
    # [core 2g cols 1:258 | core 2g+1 cols 2:257] with no boundary fixups.
    # Two full-112-partition DMAs: flat SBUF dst, (4,28,cols) DRAM src.
    agv = agout.opt().rearrange("(r p) f -> r p f", p=V)
    for h in range(2):
        o = h * TH
        nc.sync.dma_start(bg[0:P4, h, 0:TC + 1],
                          agv[0:2 * G:2, :, o + 1:o + TH])
        nc.sync.dma_start(bg[0:P4, h, TC + 1:S],
                          agv[1:2 * G:2, :, o + 2:o + TC + 1])

    # c = sum over cores of partial c; the (112 x 8) tile holds the partials
    # replicated per partition group so one reduce yields the bias directly
    csrc = agout.opt().rearrange("(r p) f -> p r f", p=V)[0:V, :,
                                                          2 * TH:2 * TH + 2]
    for g in range(G):
        nc.sync.dma_start(csb[V * g:V * g + V, :], csrc)
    nc.vector.tensor_reduce(out=cbias[:, :], in_=csb[:, :],
                            axis=mybir.AxisListType.X, op=mybir.AluOpType.add)

    # ---------------- fixed-point iterations ----------------
    # YA[28g+v, j] stores y[512g + j - 1] for j in 1..512; col 0 and col 513
    # are permanent zeros (memset once).  psum col tau = z[512g + tau] before
    # the bias; ACT writes sigmoid(psum + c) into YA[:, 1:513].
    for k in range(n_iters + 1):
        ps = psa if k % 2 == 0 else psb
        for h in range(2):
            nc.tensor.matmul(
                ps[:, :],
                lhsT=eye_sb[:, :],
                rhs=bg[:, h, :],
                start=(h == 0), stop=(k == 0 and h == 1),
            )
        if k > 0:
            nc.tensor.matmul(
                ps[:, :],
                lhsT=wmm_sb[:, 0, :],
                rhs=ya[:, 0:S],
                start=False, stop=False,
            )
            nc.tensor.matmul(
                ps[:, 0:2],
                lhsT=wmm_sb[:, 1, :],
                rhs=ya[:, S:S + 2],
                start=False, stop=True,
            )
        if k < n_iters:
            nc.scalar.activation(out=ya[:, 1:S + 1], in_=ps[:, :],
                                 func=AF.Sigmoid, bias=cbias[:, 0:1],
                                 scale=1.0)
        else:
            nc.scalar.activation(out=yfin[:, :], in_=ps[:, :],
                                 func=AF.Sigmoid, bias=cbias[:, 0:1],
                                 scale=1.0)

    # ---------------- write grouped output ----------------
    return nc.sync.dma_start(yg, yfin[:, :])


_CACHED_NC = {}


def _get_nc(reps=1):
    if reps not in _CACHED_NC:
        nc = bacc.Bacc("TRN2", target_bir_lowering=False, debug=False,
                       num_devices=N_CORES)
        MDT = F32R if USE_F32R else F32
        xt = nc.dram_tensor("xt", [128, 2 * DCH * TH], BF16,
                            kind="ExternalInput")
        w2t = nc.dram_tensor("w2t", [128, 2 * DCH * W2], BF16,
                             kind="ExternalInput")
        wmm = nc.dram_tensor("wmm", [P4, 2, P4], MDT, kind="ExternalInput")
        eye = nc.dram_tensor("eye", [P4, P4], BF16, kind="ExternalInput")
        yg = nc.dram_tensor("yg", [P4, S], F32, kind="ExternalOutput")
        with tile.TileContext(nc) as t:
            build_body(nc, xt.ap(), w2t.ap(), wmm.ap(), eye.ap(), yg.ap(),
                       tc=t, reps=reps)
        nc.compile()
        _CACHED_NC[reps] = nc
    return _CACHED_NC[reps]


def _hilo(a):
    """Split fp32 array into (hi, lo) bf16 parts: a ~ hi + lo."""
    import ml_dtypes
    hi = a.astype(ml_dtypes.bfloat16)
    lo = (a - hi.astype(np.float32)).astype(ml_dtypes.bfloat16)
    return hi, lo


def make_in_maps(x, Uo, Co, Wo):
    import ml_dtypes
    xb = np.ascontiguousarray(np.asarray(x, np.float32)[0])        # (T, D)
    w2 = np.zeros((W2, D), np.float32)
    w2[0:V] = np.asarray(Uo, np.float32)
    w2[32:32 + V] = np.asarray(Co, np.float32)
    w2tf = np.ascontiguousarray(
        w2.T.reshape(DCH, 128, W2).transpose(1, 0, 2))             # (128,8,64)
    w2h, w2l = _hilo(w2tf)
    w2t = np.ascontiguousarray(
        np.stack([w2h, w2l], axis=1).reshape(128, 2 * DCH * W2))
    wot1 = np.ascontiguousarray(np.asarray(Wo, np.float32).T)      # (V, V)
    wmm = np.zeros((P4, 2, P4), np.float32)
    for g in range(G):
        wmm[V * g:V * g + V, 0, V * g:V * g + V] = wot1
        if g > 0:
            wmm[V * (g - 1):V * (g - 1) + V, 1, V * g:V * g + V] = wot1
    eye = np.eye(P4, dtype=ml_dtypes.bfloat16)
    in_maps = []
    for r in range(N_CORES):
        xh = np.zeros((TH, D), np.float32)                         # (258, D)
        lo = r * TC - 2
        xh[max(0, -lo):, :] = xb[max(0, lo):(r + 1) * TC, :]
        xc = np.ascontiguousarray(
            xh.T.reshape(DCH, 128, TH).transpose(1, 0, 2))         # (128,8,258)
        xhi, xlo = _hilo(xc)
        xi = np.ascontiguousarray(
            np.stack([xhi, xlo], axis=1).reshape(128, 2 * DCH * TH))
        in_maps.append({"xt": xi, "w2t": w2t, "wmm": wmm, "eye": eye})
    return in_maps


def unshard_output(yg):
    y = np.empty((T, V), np.float32)
    for g in range(G):
        y[g * S:(g + 1) * S, :] = yg[V * g:V * g + V, :].T
    return y[None]


def run(inputs, trace=False, reps=1, **kw):
    nc = _get_nc(reps)
    in_maps = make_in_maps(inputs["x"], inputs["Uo"], inputs["Co"],
                           inputs["Wo"])
    res = bass_utils.run_bass_kernel_spmd(
        nc, in_maps, core_ids=list(range(N_CORES)), trace=trace, **kw)
    return unshard_output(res.results[0]["yg"]), res


def kernel(**inputs):
    out, _ = run(inputs)
    return out
